# revision 48
# baseline (speedup 1.0000x reference)
"""Causal self-attention (B=2, S=2048, E=1024, H=16) on 8 Trainium2 cores.

Sharding: core c in 0..7 handles batch b = c//4 and the 4 heads
[4*(c%4), 4*(c%4)+4).  The host pre-transposes x[b] and pre-slices the
QKV weights column-wise / Wo row-wise per core; each core computes its
heads' attention plus its partial output projection, and the host sums
the 4 partials per batch.

Device kernel (per core, everything resident in SBUF, matmul inputs in
fp16 with fp32 PSUM accumulation):
  xT [1024,2048] -> QT,KT [d,s] and V [s,d] projections, emitted as
  per-q-block "waves" interleaved into the attention stream.
  S^T tiles = matmul(lhsT=KT_blk, rhs=QT_blk): k on partitions, q on
  the free dim.  exp on ScalarE (1/sqrt(D) folded into the activation
  scale); causal masking = never computing strictly-below-diagonal
  column ranges plus one 128x128 triangular mask multiply per diagonal
  block.  P^T V is computed q-major: per (head, 128-q slice),
  matmul(lhsT=et[:, qslice], rhs=V_aug[k, 65]) accumulates O[q, d]+l
  with only 65 moving columns per k-block (vs 512 the other way
  round).  A PSUM bank supports only ONE open accumulation group, so
  each (head, q-slice) chain is emitted contiguously once its last
  k-block's exp tile exists (all of a head-pair's exp tiles stay
  SBUF-resident for the block).  The softmax denominator l is the
  ones-column of V_aug and lands as a per-partition column, so
  normalization is a [128,1] reciprocal plus a per-partition-scaled
  copy (no PE broadcast); O[q,d] tiles are PE-transposed (identity
  matmul) back to OT[d,q] for the Y = O @ Wo projection, streamed out
  per q-block as fp16 so output DMA overlaps remaining attention work.
  Projection/Y/PV chains are split into ~2-4-matmul units and woven
  between attention kb-steps by a deadline-paced fill scheduler that
  tops each block up to just above its ACT (exp) slack; input DMAs are
  split (per-chunk xT column halves, interleaved chunked weight loads)
  so the first projection matmul starts ~3us earlier and wave-0 is
  never DMA-starved.
"""

import numpy as np
from contextlib import ExitStack

B, S, E, H, D = 2, 2048, 1024, 16, 64
N_CORES = 8
CPB = 4              # cores per batch
HL = H // CPB        # heads per core = 4
DL = HL * D          # local head dims = 256
P = 128              # partitions
EC = E // P          # 8 e-chunks
SB = S // P          # 16 s/k blocks
NQB = S // 512       # 4 q blocks of 512
MT = DL // P         # 2 row-tiles of QT/KT/OT (2 heads each)

_CACHE = {}
_EXHAUSTED = object()


def _chain_gens(*gens):
    for g in gens:
        yield from g


def _emit(ctx, tc, xT, wqk, wv, wo, consts, y, loop_n=0, debug_outs=None):
    import concourse.bass as bass  # noqa: F401
    from concourse import mybir

    nc = tc.nc
    f32 = mybir.dt.float32
    f16 = mybir.dt.float16
    Exp = mybir.ActivationFunctionType.Exp

    res = ctx.enter_context(tc.tile_pool(name="res", bufs=1))
    xt_sb = res.tile([P, EC, S], f16, tag="xt")
    wqk_sb = res.tile([P, EC, 2, DL], f16, tag="wqk")
    wv_sb = res.tile([P, EC, DL], f16, tag="wv")
    wo_sb = res.tile([P, MT, E], f16, tag="wo")
    qt_sb = res.tile([P, MT, S], f16, tag="qt")
    kt_sb = res.tile([P, MT, S], f16, tag="kt")
    vt_sb = res.tile([P, SB, HL, D + 1], f16, tag="vt")
    ot_sb = res.tile([P, MT, S], f16, tag="ot")
    consts_sb = res.tile([P, 2, P], f16, tag="consts")
    mask_sb = consts_sb[:, 0, :]
    ident_sb = consts_sb[:, 1, :]

    mm_ps = ctx.enter_context(tc.tile_pool(name="mm", bufs=2, space="PSUM"))
    s_ps = ctx.enter_context(tc.tile_pool(name="sps", bufs=2, space="PSUM"))
    o_ps = ctx.enter_context(tc.tile_pool(name="ops", bufs=2, space="PSUM"))

    e_pool = ctx.enter_context(tc.tile_pool(name="ep", bufs=20))
    y_pool = ctx.enter_context(tc.tile_pool(name="yp", bufs=4))
    l_pool = ctx.enter_context(tc.tile_pool(name="lp", bufs=3))
    ob_pool = ctx.enter_context(tc.tile_pool(name="ob", bufs=4))

    def _full_body():
        dma = nc.sync

        # ---- PE pstate warm-up: the PE ramp clock starts at its first
        # instruction and runs ~2x slow for the first 3us.  Spend that
        # window on throwaway matmuls over a memset tile (result never
        # read) while the first input DMAs are still in flight, so every
        # real matmul runs at full clock.
        warm_sb = res.tile([P, P], f16, tag="warm")
        nc.vector.memset(warm_sb[:], 0.0)
        wps = mm_ps.tile([P, P], f32, tag="mm")
        for _ in range(16):
            nc.tensor.matmul(wps[:], warm_sb[:], warm_sb[:],
                             start=True, stop=True)

        # ---- loads: fine-grained so the first projection wave starts early
        # and wave-0 is never DMA-starved.  Wave-0 touches only xT columns
        # 0:512 (q-window 0 + V blocks 0..3), so those halves go first,
        # interleaved with the per-chunk q/k weight slices they pair with.
        for ec in range(EC):
            dma.dma_start(out=xt_sb[:, ec, 0:512],
                          in_=xT[ec * P:(ec + 1) * P, 0:512])
            dma.dma_start(out=wqk_sb[:, ec, :, :],
                          in_=wqk[ec * P:(ec + 1) * P, :].rearrange(
                              "p (t d) -> p t d", t=2))
            if ec == 5:
                dma.dma_start(out=wv_sb[:, 0:4, :], in_=wv[0:512, :].rearrange(
                    "(c p) d -> p c d", p=P))
        dma.dma_start(out=wv_sb[:, 4:8, :], in_=wv[512:1024, :].rearrange(
            "(c p) d -> p c d", p=P))
        dma.dma_start(out=consts_sb[:], in_=consts[:].rearrange(
            "p (t q) -> p t q", t=2))
        for ec in range(EC):
            dma.dma_start(out=xt_sb[:, ec, 512:S],
                          in_=xT[ec * P:(ec + 1) * P, 512:S])
        for dc in range(MT):
            dma.dma_start(out=wo_sb[:, dc, :], in_=wo[dc * P:(dc + 1) * P, :])
        nc.vector.memset(vt_sb[:, :, :, D:D + 1], 1.0)

        def qk_units(nb, parts=("qt", "kt")):
            # QT/KT [:, :, nb-window] = (w chunk)^T @ xT, as ~4-matmul units.
            srcs = []
            if "qt" in parts:
                srcs.append((0, qt_sb))
            if "kt" in parts:
                srcs.append((1, kt_sb))
            for mt in range(MT):
                for wi, t_sb in srcs:
                    ps = mm_ps.tile([P, 512], f32, tag="mm")
                    for ec in range(EC):
                        nc.tensor.matmul(
                            ps[:],
                            wqk_sb[:, ec, wi, mt * P:(mt + 1) * P],
                            xt_sb[:, ec, nb * 512:(nb + 1) * 512],
                            start=(ec == 0), stop=(ec == EC - 1))
                        if ec in (1, 3, 5):
                            yield
                    nc.vector.tensor_copy(
                        t_sb[:, mt, nb * 512:(nb + 1) * 512], ps[:])
                    yield

        def v_units(sb0, sb1):
            # V[sb0..sb1) = xT_blk^T @ wv, as ~4-matmul units.
            for sb in range(sb0, sb1):
                ps = mm_ps.tile([P, 512], f32, tag="mm")
                for ec in range(EC):
                    nc.tensor.matmul(
                        ps[:, 0:DL],
                        xt_sb[:, ec, sb * P:(sb + 1) * P],
                        wv_sb[:, ec, :],
                        start=(ec == 0), stop=(ec == EC - 1))
                    if ec == 3:
                        yield
                nc.vector.tensor_copy(
                    vt_sb[:, sb, :, 0:D],
                    ps[:, 0:DL].rearrange("p (h d) -> p h d", h=HL))
                yield

        def out_proj_units(qb, act_copies=False):
            # Y[sb, :] = O[sb, :] @ wo for this q-block's 4 s-blocks; each
            # 512-wide half is copied fp16 and DMA'd immediately so the
            # final copy->DMA tail stays short.  act_copies splits the
            # PSUM->SBUF copies across DVE and the (by then idle) ACT.
            for sb in range(4 * qb, 4 * qb + 4):
                yt = y_pool.tile([P, E], f16, tag="y")
                for eb in range(E // 512):
                    yp = mm_ps.tile([P, 512], f32, tag="mm")
                    for dc in range(MT):
                        nc.tensor.matmul(
                            yp[:],
                            ot_sb[:, dc, sb * P:(sb + 1) * P],
                            wo_sb[:, dc, eb * 512:(eb + 1) * 512],
                            start=(dc == 0), stop=(dc == MT - 1))
                    dst = yt[:, eb * 512:(eb + 1) * 512]
                    if act_copies and eb == 1:
                        nc.scalar.copy(dst, yp[:])
                    else:
                        nc.vector.tensor_copy(dst, yp[:])
                    dma.dma_start(
                        out=y[sb * P:(sb + 1) * P, eb * 512:(eb + 1) * 512],
                        in_=dst)
                    yield

        def pv_norm_units(qb, mt, qs, ets):
            # P^T V for one (head-pair, q-slice): two contiguous
            # accumulation chains (one per head, each alone in its PSUM
            # bank -- a bank supports only ONE open accumulation group at
            # a time), then per-partition normalize by the ones-column l
            # (reciprocal + scaled copies on DVE; ACT would inflate the
            # counting-semaphore thresholds every exp-wait uses) and a PE
            # transpose (identity matmul) back to OT[d,q].
            last = 4 * qb + qs
            ohs = []
            for half in range(2):
                oh = o_ps.tile([P, D + 1], f32, tag="o")
                for kb in range(last + 1):
                    nc.tensor.matmul(
                        oh[:],
                        ets[kb][:, half * 512 + qs * P:
                                half * 512 + (qs + 1) * P],
                        vt_sb[:, kb, 2 * mt + half, :],
                        start=(kb == 0), stop=(kb == last))
                ohs.append(oh)
                yield
            ob = ob_pool.tile([P, P], f16, tag="ob")
            for half in range(2):
                rec = l_pool.tile([P, 1], f32, tag="rec")
                nc.vector.reciprocal(rec[:], ohs[half][:, D:D + 1])
                nc.vector.tensor_scalar_mul(
                    ob[:, half * D:(half + 1) * D],
                    ohs[half][:, 0:D], rec[:])
            tr = mm_ps.tile([P, P], f16, tag="mm")
            nc.tensor.transpose(tr[:], ob[:], ident_sb)
            q0 = qb * 512 + qs * P
            nc.vector.tensor_copy(ot_sb[:, mt, q0:q0 + P], tr[:])
            yield

        def attention_block(qb, fills):
            # ACT-paced; fill units (wave / Y chains, ~2-4 matmuls each) are
            # emitted around the PV batch of each kb-step, so PE chews fill
            # while ACT runs exp.  `fills` is a list of [gen, total, done,
            # deadline-substep] streams, paced linearly toward each
            # deadline; leftovers are returned for the next block.
            nkb = 4 * (qb + 1)     # causal: k blocks 0 .. nkb-1
            scale = float(1.0 / np.sqrt(D))
            nsteps = MT * nkb
            fill_steps = 2 * nsteps
            for f in fills:
                if f[3] is None:
                    f[3] = fill_steps
                if len(f) < 5:
                    f.append(0)

            def run_fill(substep):
                for f in fills:
                    gen, total, done, dl, st = f
                    if done >= total or substep < st:
                        continue
                    span = max(1, dl - st)
                    want = min(total, -((-total * min(substep + 1 - st,
                                                      span)) // span))
                    while f[2] < want:
                        if next(gen, _EXHAUSTED) is _EXHAUSTED:
                            f[2] = total
                            break
                        f[2] += 1

            run_fill(0)   # pre-fill: cover the inter-block exp catch-up
            step = 0
            for mt in range(MT):   # head pair (2*mt, 2*mt+1)
                ets = []           # this head-pair's exp tiles, kept live
                                   # until their last PV chain reads them
                for kb in range(nkb):
                    t = kb - 4 * qb
                    v0 = P * t if t > 0 else 0   # masked prefix of window
                    sp = s_ps.tile([P, 1024], f32, tag="s")
                    for half in range(2):
                        dr = half * D
                        nc.tensor.matmul(
                            sp[:, half * 512 + v0:(half + 1) * 512],
                            kt_sb[dr:dr + D, mt, kb * P:(kb + 1) * P],
                            qt_sb[dr:dr + D, mt, qb * 512 + v0:(qb + 1) * 512],
                            start=True, stop=True)
                    et = e_pool.tile([P, 1024], f16, tag="e")
                    ets.append(et)
                    nc.scalar.activation(out=et[:, v0:], in_=sp[:, v0:],
                                         func=Exp, scale=scale)
                    if t >= 0:  # diagonal block: mask strictly-future keys
                        for half in range(2):
                            w0 = half * 512 + v0
                            nc.vector.tensor_mul(
                                et[:, w0:w0 + P], et[:, w0:w0 + P], mask_sb)
                        # all et for q-slice qs=t now exist: its PV chains
                        # become fill, run asap (frees o banks quickly)
                        fills.insert(
                            0, [pv_norm_units(qb, mt, t, ets), 3, 0, 1, 0])
                    run_fill(2 * step)  # PE fill while ACT runs this exp
                    run_fill(2 * step + 1)
                    step += 1
            left = [f for f in fills if f[2] < f[1]]
            for f in left:      # re-spread non-asap leftovers next block
                if f[3] != 1:
                    f[3] = None
                    f[4] = 0
            return left

        # wave(0): the four q/k window-0 chains interleaved per e-chunk
        # (two accumulators from the mm pool, two borrowed from the still
        # idle o pool), then V(0..5) -- paced behind the xT column-half
        # and weight-chunk DMAs issued first.
        w0ps = []
        for mt in range(MT):
            pq = mm_ps.tile([P, 512], f32, tag="mm")
            pk = o_ps.tile([P, 512], f32, tag="o")
            w0ps.append((mt, 0, qt_sb, pq))
            w0ps.append((mt, 1, kt_sb, pk))
        for ec in range(EC):
            for mt, wi, t_sb, pchain in w0ps:
                nc.tensor.matmul(
                    pchain[:],
                    wqk_sb[:, ec, wi, mt * P:(mt + 1) * P],
                    xt_sb[:, ec, 0:512],
                    start=(ec == 0), stop=(ec == EC - 1))
        for mt, wi, t_sb, pchain in w0ps:
            nc.vector.tensor_copy(t_sb[:, mt, 0:512], pchain[:])
        for _ in v_units(0, 6):
            pass
        # Fill plan: every attention block is topped up to just above its
        # ACT (exp) slack so no block ends PE-dry; kt/V windows carry
        # substep deadlines = just before the S/PV that first reads them.
        plan = [
            [[qk_units(1, ("qt",)), 8, 0, None, 0],
             [v_units(6, 8), 4, 0, None, 0]],
            [[qk_units(1, ("kt",)), 8, 0, 6, 0],
             [qk_units(2, ("qt",)), 8, 0, None, 0],
             [v_units(8, 12), 8, 0, None, 0]],
            [[qk_units(2, ("kt",)), 8, 0, 14, 0],
             [qk_units(3, ("qt",)), 8, 0, None, 0],
             [out_proj_units(0), 8, 0, None, 16]],
            [[qk_units(3, ("kt",)), 8, 0, 22, 0],
             [v_units(12, 16), 8, 0, 24, 0],
             [out_proj_units(1), 8, 0, None, 24],
             [out_proj_units(2), 8, 0, None, 40],
             [out_proj_units(3, act_copies=True), 8, 0, None, 56]],
        ]
        carry = []
        for qb in range(NQB):
            carry = attention_block(qb, carry + plan[qb])
        for f in carry:
            while next(f[0], _EXHAUSTED) is not _EXHAUSTED:
                pass
        if debug_outs:
            dma.dma_start(out=debug_outs["qt"],
                          in_=qt_sb.rearrange("p a b -> p (a b)"))
            dma.dma_start(out=debug_outs["kt"],
                          in_=kt_sb.rearrange("p a b -> p (a b)"))
            dma.dma_start(out=debug_outs["vt"],
                          in_=vt_sb.rearrange("p a b c -> p (a b c)"))
            dma.dma_start(out=debug_outs["ot"],
                          in_=ot_sb.rearrange("p a b -> p (a b)"))

    if loop_n:
        # bench-only path: hint all engines so the back-edge prefetches
        # the body's IRAM blocks (body >256 instructions per engine)
        hints = (mybir.EngineType.PE, mybir.EngineType.Activation,
                 mybir.EngineType.DVE, mybir.EngineType.SP,
                 mybir.EngineType.Pool)
        with tc.For_i(0, loop_n, 1, hint_engines=hints):
            _full_body()
    else:
        _full_body()


def _get_program(loop_n=0):
    key = ("nc", loop_n)
    if key in _CACHE:
        return _CACHE[key]
    import concourse.tile as tile
    from concourse import bacc, mybir

    f16 = mybir.dt.float16
    nc = bacc.Bacc("TRN2", target_bir_lowering=False, debug=False,
                   enable_asserts=False)
    xT = nc.dram_tensor("xT", [E, S], f16, kind="ExternalInput").ap()
    wqk = nc.dram_tensor("wqk", [E, 2 * DL], f16, kind="ExternalInput").ap()
    wv = nc.dram_tensor("wv", [E, DL], f16, kind="ExternalInput").ap()
    wo = nc.dram_tensor("wo", [DL, E], f16, kind="ExternalInput").ap()
    consts = nc.dram_tensor("consts", [P, 2 * P], f16,
                            kind="ExternalInput").ap()
    y = nc.dram_tensor("y", [S, E], f16, kind="ExternalOutput").ap()
    with tile.TileContext(nc) as tc:
        with ExitStack() as ctx:
            _emit(ctx, tc, xT, wqk, wv, wo, consts, y, loop_n=loop_n)
    nc.compile()
    _CACHE[key] = nc
    return nc


def _make_in_maps(x, Wq, Wk, Wv, Wo):
    x = np.asarray(x, dtype=np.float32)
    Wq = np.asarray(Wq, dtype=np.float32)
    Wk = np.asarray(Wk, dtype=np.float32)
    Wv = np.asarray(Wv, dtype=np.float32)
    Wo = np.asarray(Wo, dtype=np.float32)
    consts = np.concatenate(
        [np.triu(np.ones((P, P), dtype=np.float16)),
         np.eye(P, dtype=np.float16)], axis=1)
    in_maps = []
    for c in range(N_CORES):
        b, hg = divmod(c, CPB)
        hs = slice(hg * HL, (hg + 1) * HL)
        wq_l = Wq.reshape(E, H, D)[:, hs, :].reshape(E, DL)
        wk_l = Wk.reshape(E, H, D)[:, hs, :].reshape(E, DL)
        in_maps.append({
            "xT": np.ascontiguousarray(x[b].T).astype(np.float16),
            "wqk": np.ascontiguousarray(
                np.concatenate([wq_l, wk_l], axis=1)).astype(np.float16),
            "wv": np.ascontiguousarray(
                Wv.reshape(E, H, D)[:, hs, :].reshape(E, DL)).astype(
                    np.float16),
            "wo": np.ascontiguousarray(
                Wo.reshape(H, D, E)[hs, :, :].reshape(DL, E)).astype(
                    np.float16),
            "consts": consts,
        })
    return in_maps


def run(x, Wq, Wk, Wv, Wo, trace=False):
    from concourse.bass_utils import run_bass_kernel_spmd

    nc = _get_program()
    in_maps = _make_in_maps(x, Wq, Wk, Wv, Wo)
    br = run_bass_kernel_spmd(nc, in_maps, list(range(N_CORES)), trace=trace)
    out = np.zeros((B, S, E), dtype=np.float32)
    for c in range(N_CORES):
        out[c // CPB] += br.results[c]["y"].astype(np.float32)
    return out, br


def kernel(x, Wq, Wk, Wv, Wo):
    out, _ = run(x, Wq, Wk, Wv, Wo, trace=False)
    return out


# revision 49
# speedup vs baseline: 1.0046x; 1.0046x over previous
"""Causal self-attention (B=2, S=2048, E=1024, H=16) on 8 Trainium2 cores.

Sharding: core c in 0..7 handles batch b = c//4 and the 4 heads
[4*(c%4), 4*(c%4)+4).  The host pre-transposes x[b] and pre-slices the
QKV weights column-wise / Wo row-wise per core; each core computes its
heads' attention plus its partial output projection, and the host sums
the 4 partials per batch.

Device kernel (per core, everything resident in SBUF, matmul inputs in
fp16 with fp32 PSUM accumulation):
  xT [1024,2048] -> QT,KT [d,s] and V [s,d] projections, emitted as
  per-q-block "waves" interleaved into the attention stream.
  S^T tiles = matmul(lhsT=KT_blk, rhs=QT_blk): k on partitions, q on
  the free dim.  exp on ScalarE (1/sqrt(D) folded into the activation
  scale); causal masking = never computing strictly-below-diagonal
  column ranges plus one 128x128 triangular mask multiply per diagonal
  block.  P^T V is computed q-major: per (head, 128-q slice),
  matmul(lhsT=et[:, qslice], rhs=V_aug[k, 65]) accumulates O[q, d]+l
  with only 65 moving columns per k-block (vs 512 the other way
  round).  A PSUM bank supports only ONE open accumulation group, so
  each (head, q-slice) chain is emitted contiguously once its last
  k-block's exp tile exists (all of a head-pair's exp tiles stay
  SBUF-resident for the block).  The softmax denominator l is the
  ones-column of V_aug and lands as a per-partition column, so
  normalization is a [128,1] reciprocal plus a per-partition-scaled
  copy (no PE broadcast); O[q,d] tiles are PE-transposed (identity
  matmul) back to OT[d,q] for the Y = O @ Wo projection, streamed out
  per q-block as fp16 so output DMA overlaps remaining attention work.
  Projection/Y/PV chains are split into ~2-4-matmul units and woven
  between attention kb-steps by a deadline-paced fill scheduler that
  tops each block up to just above its ACT (exp) slack; input DMAs are
  split (per-chunk xT column halves, interleaved chunked weight loads)
  so the first projection matmul starts ~3us earlier and wave-0 is
  never DMA-starved.
"""

import numpy as np
from contextlib import ExitStack

B, S, E, H, D = 2, 2048, 1024, 16, 64
N_CORES = 8
CPB = 4              # cores per batch
HL = H // CPB        # heads per core = 4
DL = HL * D          # local head dims = 256
P = 128              # partitions
EC = E // P          # 8 e-chunks
SB = S // P          # 16 s/k blocks
NQB = S // 512       # 4 q blocks of 512
MT = DL // P         # 2 row-tiles of QT/KT/OT (2 heads each)

_CACHE = {}
_EXHAUSTED = object()


def _chain_gens(*gens):
    for g in gens:
        yield from g


def _emit(ctx, tc, xT, wqk, wv, wo, consts, boot, y, loop_n=0,
          debug_outs=None):
    import concourse.bass as bass  # noqa: F401
    from concourse import mybir

    nc = tc.nc
    f32 = mybir.dt.float32
    f16 = mybir.dt.float16
    Exp = mybir.ActivationFunctionType.Exp

    res = ctx.enter_context(tc.tile_pool(name="res", bufs=1))
    xt_sb = res.tile([P, EC, S], f16, tag="xt")
    wqk_sb = res.tile([P, EC, 2, DL], f16, tag="wqk")
    wv_sb = res.tile([P, EC, DL], f16, tag="wv")
    wo_sb = res.tile([P, MT, E], f16, tag="wo")
    qt_sb = res.tile([P, MT, S], f16, tag="qt")
    kt_sb = res.tile([P, MT, S], f16, tag="kt")
    vt_sb = res.tile([P, SB, HL, D + 1], f16, tag="vt")
    ot_sb = res.tile([P, MT, S], f16, tag="ot")
    consts_sb = res.tile([P, 2, P], f16, tag="consts")
    boot_sb = res.tile([P, 4 * 256], f16, tag="boot")
    mask_sb = consts_sb[:, 0, :]
    ident_sb = consts_sb[:, 1, :]

    mm_ps = ctx.enter_context(tc.tile_pool(name="mm", bufs=2, space="PSUM"))
    s_ps = ctx.enter_context(tc.tile_pool(name="sps", bufs=2, space="PSUM"))
    o_ps = ctx.enter_context(tc.tile_pool(name="ops", bufs=2, space="PSUM"))

    e_pool = ctx.enter_context(tc.tile_pool(name="ep", bufs=20))
    y_pool = ctx.enter_context(tc.tile_pool(name="yp", bufs=4))
    l_pool = ctx.enter_context(tc.tile_pool(name="lp", bufs=3))
    ob_pool = ctx.enter_context(tc.tile_pool(name="ob", bufs=4))

    def _full_body():
        dma = nc.sync

        # ---- PE pstate warm-up: the PE ramp clock starts at its first
        # instruction and runs ~2x slow for the first 3us.  Spend that
        # window on throwaway matmuls over a memset tile (result never
        # read) while the first input DMAs are still in flight, so every
        # real matmul runs at full clock.
        warm_sb = res.tile([P, P], f16, tag="warm")
        nc.vector.memset(warm_sb[:], 0.0)
        wps = mm_ps.tile([P, P], f32, tag="mm")
        for _ in range(16):
            nc.tensor.matmul(wps[:], warm_sb[:], warm_sb[:],
                             start=True, stop=True)

        # ---- loads: fine-grained so the first projection wave starts early
        # and wave-0 is never DMA-starved.  Wave-0 touches only xT columns
        # 0:512 (q-window 0 + V blocks 0..3), so those halves go first,
        # interleaved with the per-chunk q/k weight slices they pair with.
        # ec0's xT half + q/k weights ride one host-concatenated "boot"
        # DMA so the first matmul waits on a single transfer.
        dma.dma_start(out=boot_sb[:], in_=boot[:])
        for ec in range(EC):
            if ec > 0:
                dma.dma_start(out=xt_sb[:, ec, 0:512],
                              in_=xT[ec * P:(ec + 1) * P, 0:512])
                dma.dma_start(out=wqk_sb[:, ec, :, :],
                              in_=wqk[ec * P:(ec + 1) * P, :].rearrange(
                                  "p (t d) -> p t d", t=2))
            if ec == 5:
                dma.dma_start(out=wv_sb[:, 0:4, :], in_=wv[0:512, :].rearrange(
                    "(c p) d -> p c d", p=P))
        dma.dma_start(out=wv_sb[:, 4:8, :], in_=wv[512:1024, :].rearrange(
            "(c p) d -> p c d", p=P))
        dma.dma_start(out=consts_sb[:], in_=consts[:].rearrange(
            "p (t q) -> p t q", t=2))
        for ec in range(EC):
            dma.dma_start(out=xt_sb[:, ec, 512:S],
                          in_=xT[ec * P:(ec + 1) * P, 512:S])
        for dc in range(MT):
            dma.dma_start(out=wo_sb[:, dc, :], in_=wo[dc * P:(dc + 1) * P, :])
        nc.vector.memset(vt_sb[:, :, :, D:D + 1], 1.0)

        def xt_cols(ec, c0, c1):
            if ec == 0 and c1 <= 512:
                return boot_sb[:, c0:c1]
            return xt_sb[:, ec, c0:c1]

        def wqk_block(ec, wi, mt):
            if ec == 0:
                b0 = 512 + wi * 256 + mt * P
                return boot_sb[:, b0:b0 + P]
            return wqk_sb[:, ec, wi, mt * P:(mt + 1) * P]

        def qk_units(nb, parts=("qt", "kt")):
            # QT/KT [:, :, nb-window] = (w chunk)^T @ xT, as ~4-matmul units.
            srcs = []
            if "qt" in parts:
                srcs.append((0, qt_sb))
            if "kt" in parts:
                srcs.append((1, kt_sb))
            for mt in range(MT):
                for wi, t_sb in srcs:
                    ps = mm_ps.tile([P, 512], f32, tag="mm")
                    for ec in range(EC):
                        nc.tensor.matmul(
                            ps[:],
                            wqk_block(ec, wi, mt),
                            xt_sb[:, ec, nb * 512:(nb + 1) * 512],
                            start=(ec == 0), stop=(ec == EC - 1))
                        if ec in (1, 3, 5):
                            yield
                    nc.vector.tensor_copy(
                        t_sb[:, mt, nb * 512:(nb + 1) * 512], ps[:])
                    yield

        def v_units(sb0, sb1):
            # V[sb0..sb1) = xT_blk^T @ wv, as ~4-matmul units.
            for sb in range(sb0, sb1):
                ps = mm_ps.tile([P, 512], f32, tag="mm")
                for ec in range(EC):
                    nc.tensor.matmul(
                        ps[:, 0:DL],
                        xt_cols(ec, sb * P, (sb + 1) * P),
                        wv_sb[:, ec, :],
                        start=(ec == 0), stop=(ec == EC - 1))
                    if ec == 3:
                        yield
                nc.vector.tensor_copy(
                    vt_sb[:, sb, :, 0:D],
                    ps[:, 0:DL].rearrange("p (h d) -> p h d", h=HL))
                yield

        def out_proj_units(qb, act_copies=False):
            # Y[sb, :] = O[sb, :] @ wo for this q-block's 4 s-blocks; each
            # 512-wide half is copied fp16 and DMA'd immediately so the
            # final copy->DMA tail stays short.  act_copies splits the
            # PSUM->SBUF copies across DVE and the (by then idle) ACT.
            for sb in range(4 * qb, 4 * qb + 4):
                yt = y_pool.tile([P, E], f16, tag="y")
                for eb in range(E // 512):
                    yp = mm_ps.tile([P, 512], f32, tag="mm")
                    for dc in range(MT):
                        nc.tensor.matmul(
                            yp[:],
                            ot_sb[:, dc, sb * P:(sb + 1) * P],
                            wo_sb[:, dc, eb * 512:(eb + 1) * 512],
                            start=(dc == 0), stop=(dc == MT - 1))
                    dst = yt[:, eb * 512:(eb + 1) * 512]
                    if act_copies and eb == 1:
                        nc.scalar.copy(dst, yp[:])
                    else:
                        nc.vector.tensor_copy(dst, yp[:])
                    dma.dma_start(
                        out=y[sb * P:(sb + 1) * P, eb * 512:(eb + 1) * 512],
                        in_=dst)
                    yield

        def pv_norm_units(qb, mt, qs, ets):
            # P^T V for one (head-pair, q-slice): two contiguous
            # accumulation chains (one per head, each alone in its PSUM
            # bank -- a bank supports only ONE open accumulation group at
            # a time), then per-partition normalize by the ones-column l
            # (reciprocal + scaled copies on DVE; ACT would inflate the
            # counting-semaphore thresholds every exp-wait uses) and a PE
            # transpose (identity matmul) back to OT[d,q].
            last = 4 * qb + qs
            ohs = []
            for half in range(2):
                oh = o_ps.tile([P, D + 1], f32, tag="o")
                for kb in range(last + 1):
                    nc.tensor.matmul(
                        oh[:],
                        ets[kb][:, half * 512 + qs * P:
                                half * 512 + (qs + 1) * P],
                        vt_sb[:, kb, 2 * mt + half, :],
                        start=(kb == 0), stop=(kb == last))
                ohs.append(oh)
                yield
            ob = ob_pool.tile([P, P], f16, tag="ob")
            for half in range(2):
                rec = l_pool.tile([P, 1], f32, tag="rec")
                nc.vector.reciprocal(rec[:], ohs[half][:, D:D + 1])
                nc.vector.tensor_scalar_mul(
                    ob[:, half * D:(half + 1) * D],
                    ohs[half][:, 0:D], rec[:])
            tr = mm_ps.tile([P, P], f16, tag="mm")
            nc.tensor.transpose(tr[:], ob[:], ident_sb)
            q0 = qb * 512 + qs * P
            nc.vector.tensor_copy(ot_sb[:, mt, q0:q0 + P], tr[:])
            yield

        def attention_block(qb, fills):
            # ACT-paced; fill units (wave / Y chains, ~2-4 matmuls each) are
            # emitted around the PV batch of each kb-step, so PE chews fill
            # while ACT runs exp.  `fills` is a list of [gen, total, done,
            # deadline-substep] streams, paced linearly toward each
            # deadline; leftovers are returned for the next block.
            nkb = 4 * (qb + 1)     # causal: k blocks 0 .. nkb-1
            scale = float(1.0 / np.sqrt(D))
            nsteps = MT * nkb
            fill_steps = 2 * nsteps
            for f in fills:
                if f[3] is None:
                    f[3] = fill_steps
                if len(f) < 5:
                    f.append(0)

            def run_fill(substep):
                for f in fills:
                    gen, total, done, dl, st = f
                    if done >= total or substep < st:
                        continue
                    span = max(1, dl - st)
                    want = min(total, -((-total * min(substep + 1 - st,
                                                      span)) // span))
                    while f[2] < want:
                        if next(gen, _EXHAUSTED) is _EXHAUSTED:
                            f[2] = total
                            break
                        f[2] += 1

            run_fill(0)   # pre-fill: cover the inter-block exp catch-up
            step = 0
            for mt in range(MT):   # head pair (2*mt, 2*mt+1)
                ets = []           # this head-pair's exp tiles, kept live
                                   # until their last PV chain reads them
                for kb in range(nkb):
                    t = kb - 4 * qb
                    v0 = P * t if t > 0 else 0   # masked prefix of window
                    sp = s_ps.tile([P, 1024], f32, tag="s")
                    for half in range(2):
                        dr = half * D
                        nc.tensor.matmul(
                            sp[:, half * 512 + v0:(half + 1) * 512],
                            kt_sb[dr:dr + D, mt, kb * P:(kb + 1) * P],
                            qt_sb[dr:dr + D, mt, qb * 512 + v0:(qb + 1) * 512],
                            start=True, stop=True)
                    et = e_pool.tile([P, 1024], f16, tag="e")
                    ets.append(et)
                    nc.scalar.activation(out=et[:, v0:], in_=sp[:, v0:],
                                         func=Exp, scale=scale)
                    if t >= 0:  # diagonal block: mask strictly-future keys
                        for half in range(2):
                            w0 = half * 512 + v0
                            nc.vector.tensor_mul(
                                et[:, w0:w0 + P], et[:, w0:w0 + P], mask_sb)
                        # all et for q-slice qs=t now exist: its PV chains
                        # become fill, run asap (frees o banks quickly)
                        fills.insert(
                            0, [pv_norm_units(qb, mt, t, ets), 3, 0, 1, 0])
                    run_fill(2 * step)  # PE fill while ACT runs this exp
                    run_fill(2 * step + 1)
                    step += 1
            left = [f for f in fills if f[2] < f[1]]
            for f in left:      # re-spread non-asap leftovers next block
                if f[3] != 1:
                    f[3] = None
                    f[4] = 0
            return left

        # wave(0): the four q/k window-0 chains interleaved per e-chunk
        # (two accumulators from the mm pool, two borrowed from the still
        # idle o pool), then V(0..5) -- paced behind the xT column-half
        # and weight-chunk DMAs issued first.
        w0ps = []
        for mt in range(MT):
            pq = mm_ps.tile([P, 512], f32, tag="mm")
            pk = o_ps.tile([P, 512], f32, tag="o")
            w0ps.append((mt, 0, qt_sb, pq))
            w0ps.append((mt, 1, kt_sb, pk))
        for ec in range(EC):
            for mt, wi, t_sb, pchain in w0ps:
                nc.tensor.matmul(
                    pchain[:],
                    wqk_block(ec, wi, mt),
                    xt_cols(ec, 0, 512),
                    start=(ec == 0), stop=(ec == EC - 1))
        for mt, wi, t_sb, pchain in w0ps:
            nc.vector.tensor_copy(t_sb[:, mt, 0:512], pchain[:])
        for _ in v_units(0, 6):
            pass
        # Fill plan: every attention block is topped up to just above its
        # ACT (exp) slack so no block ends PE-dry; kt/V windows carry
        # substep deadlines = just before the S/PV that first reads them.
        plan = [
            [[qk_units(1, ("qt",)), 8, 0, None, 0],
             [v_units(6, 8), 4, 0, None, 0]],
            [[qk_units(1, ("kt",)), 8, 0, 6, 0],
             [qk_units(2, ("qt",)), 8, 0, None, 0],
             [v_units(8, 12), 8, 0, None, 0]],
            [[qk_units(2, ("kt",)), 8, 0, 14, 0],
             [qk_units(3, ("qt",)), 8, 0, None, 0],
             [out_proj_units(0), 8, 0, None, 16]],
            [[qk_units(3, ("kt",)), 8, 0, 22, 0],
             [v_units(12, 16), 8, 0, 24, 0],
             [out_proj_units(1), 8, 0, None, 24],
             [out_proj_units(2), 8, 0, None, 40],
             [out_proj_units(3, act_copies=True), 8, 0, None, 56]],
        ]
        carry = []
        for qb in range(NQB):
            carry = attention_block(qb, carry + plan[qb])
        for f in carry:
            while next(f[0], _EXHAUSTED) is not _EXHAUSTED:
                pass
        if debug_outs:
            dma.dma_start(out=debug_outs["qt"],
                          in_=qt_sb.rearrange("p a b -> p (a b)"))
            dma.dma_start(out=debug_outs["kt"],
                          in_=kt_sb.rearrange("p a b -> p (a b)"))
            dma.dma_start(out=debug_outs["vt"],
                          in_=vt_sb.rearrange("p a b c -> p (a b c)"))
            dma.dma_start(out=debug_outs["ot"],
                          in_=ot_sb.rearrange("p a b -> p (a b)"))

    if loop_n:
        # bench-only path: hint all engines so the back-edge prefetches
        # the body's IRAM blocks (body >256 instructions per engine)
        hints = (mybir.EngineType.PE, mybir.EngineType.Activation,
                 mybir.EngineType.DVE, mybir.EngineType.SP,
                 mybir.EngineType.Pool)
        with tc.For_i(0, loop_n, 1, hint_engines=hints):
            _full_body()
    else:
        _full_body()


def _get_program(loop_n=0):
    key = ("nc", loop_n)
    if key in _CACHE:
        return _CACHE[key]
    import concourse.tile as tile
    from concourse import bacc, mybir

    f16 = mybir.dt.float16
    nc = bacc.Bacc("TRN2", target_bir_lowering=False, debug=False,
                   enable_asserts=False)
    xT = nc.dram_tensor("xT", [E, S], f16, kind="ExternalInput").ap()
    wqk = nc.dram_tensor("wqk", [E, 2 * DL], f16, kind="ExternalInput").ap()
    wv = nc.dram_tensor("wv", [E, DL], f16, kind="ExternalInput").ap()
    wo = nc.dram_tensor("wo", [DL, E], f16, kind="ExternalInput").ap()
    consts = nc.dram_tensor("consts", [P, 2 * P], f16,
                            kind="ExternalInput").ap()
    boot = nc.dram_tensor("boot", [P, 4 * 256], f16,
                          kind="ExternalInput").ap()
    y = nc.dram_tensor("y", [S, E], f16, kind="ExternalOutput").ap()
    with tile.TileContext(nc) as tc:
        with ExitStack() as ctx:
            _emit(ctx, tc, xT, wqk, wv, wo, consts, boot, y,
                  loop_n=loop_n)
    nc.compile()
    _CACHE[key] = nc
    return nc


def _make_in_maps(x, Wq, Wk, Wv, Wo):
    x = np.asarray(x, dtype=np.float32)
    Wq = np.asarray(Wq, dtype=np.float32)
    Wk = np.asarray(Wk, dtype=np.float32)
    Wv = np.asarray(Wv, dtype=np.float32)
    Wo = np.asarray(Wo, dtype=np.float32)
    consts = np.concatenate(
        [np.triu(np.ones((P, P), dtype=np.float16)),
         np.eye(P, dtype=np.float16)], axis=1)
    in_maps = []
    for c in range(N_CORES):
        b, hg = divmod(c, CPB)
        hs = slice(hg * HL, (hg + 1) * HL)
        wq_l = Wq.reshape(E, H, D)[:, hs, :].reshape(E, DL)
        wk_l = Wk.reshape(E, H, D)[:, hs, :].reshape(E, DL)
        xT_np = np.ascontiguousarray(x[b].T).astype(np.float16)
        wqk_np = np.ascontiguousarray(
            np.concatenate([wq_l, wk_l], axis=1)).astype(np.float16)
        in_maps.append({
            "xT": xT_np,
            "wqk": wqk_np,
            "boot": np.ascontiguousarray(np.concatenate(
                [xT_np[0:P, 0:512], wqk_np[0:P, :]], axis=1)),
            "wv": np.ascontiguousarray(
                Wv.reshape(E, H, D)[:, hs, :].reshape(E, DL)).astype(
                    np.float16),
            "wo": np.ascontiguousarray(
                Wo.reshape(H, D, E)[hs, :, :].reshape(DL, E)).astype(
                    np.float16),
            "consts": consts,
        })
    return in_maps


def run(x, Wq, Wk, Wv, Wo, trace=False):
    from concourse.bass_utils import run_bass_kernel_spmd

    nc = _get_program()
    in_maps = _make_in_maps(x, Wq, Wk, Wv, Wo)
    br = run_bass_kernel_spmd(nc, in_maps, list(range(N_CORES)), trace=trace)
    out = np.zeros((B, S, E), dtype=np.float32)
    for c in range(N_CORES):
        out[c // CPB] += br.results[c]["y"].astype(np.float32)
    return out, br


def kernel(x, Wq, Wk, Wv, Wo):
    out, _ = run(x, Wq, Wk, Wv, Wo, trace=False)
    return out
